# revision 17
# baseline (speedup 1.0000x reference)
# Trainium2 Bass kernel for CausalSelfAttention (B=2, T=2048, C=1024, H=16, D=64)
# with periodic mask: causal AND (key_col % 4 != 3).
#
# Sharding (8 NeuronCores): core c = (b, g) with b = c//4 (batch), g = c%4
# (head group of 4 heads). Each core computes QKV for its 4 heads, attention,
# and a partial output projection y_heads @ Wp[rows]. Host sums the 4 partials
# per batch and adds bp (tensor-parallel reduce).
#
# Key device-side choices:
#  - All DRAM inputs are host-prepacked to [128, ...] partition-major layouts
#    so every DMA descriptor moves 4-8KB contiguous runs (fewer descriptors,
#    less issuing-engine busy time, faster rings).
#  - DMAs are emitted in first-use order, k-granular for the j=0 window, so
#    the first QKV matmul starts as early as possible and the PE never goes
#    HAM-cold once started.
#  - The periodic mask is exploited as compaction: keys at t%4==3 are never
#    attended. K^T is computed directly at kept columns via a 3D moving AP
#    (384-wide matmuls); V is computed full then row-gathered via 0/1
#    selection matmuls.
#  - Scores are produced transposed (S^T[tk_kept, tq]); the two heads of a
#    row-group pair are issued back-to-back so their K=64 matmuls execute
#    CONCURRENTLY on disjoint PE row groups (h0/h64). AV of tile i-1 is
#    emitted after the score pair of tile i to keep pairs adjacent in the
#    static schedule.
#  - Softmax row sums: V tiles carry a 64-wide all-ones block (memset on
#    device), so each AV accumulation produces [64 x y ; 64 x row-sums] in
#    PSUM; a DVE fast-reciprocal + multiply normalizes during PSUM->SBUF.
#  - The projection path (yt, Wp, stage, out) runs in bf16: halves the
#    output DMA and copy costs at ~1e-3 extra relative error.

import ml_dtypes
import numpy as np

B, T, C, H, D = 2, 2048, 1024, 16, 64
HG = 4          # heads per core
CG = HG * D     # = 256 columns of C per core
TK = (T // 4) * 3   # 1536 kept key positions
NTK = TK // 128     # 12 kept-key chunks of 128
SCALE = 1.0 / 8.0   # 1/sqrt(D)

_CACHE = {}


def _split_multi_waits(nc, mybir):
    # The pinned walrus here encodes at most 1 sync-wait per instruction
    # (2 for EventSemaphore). Hoist excess waits onto standalone NoOps that
    # precede the instruction on the same engine.
    f = nc.m.functions[0]
    n = 0
    for b in f.blocks:
        insts = list(b.instructions)
        out = []
        changed = False
        for inst in insts:
            si = inst.sync_info
            if si is not None:
                waits = list(si.on_wait)
                cap = 2 if isinstance(inst, mybir.InstEventSemaphore) else 1
                if len(waits) > cap:
                    for w in waits[cap:]:
                        out.append(mybir.InstNoOp(
                            name=f"{inst.name}-ws{n}", engine=inst.engine,
                            ins=[], outs=[],
                            sync_info=mybir.SyncInfo(on_wait=[w], on_update=[])))
                        n += 1
                    inst.sync_info = mybir.SyncInfo(
                        on_wait=waits[:cap], on_update=list(si.on_update))
                    changed = True
            out.append(inst)
        if changed:
            b.instructions = out
    return n


def _build_bass(split=True):
    import concourse.bass as bass
    import concourse.tile as tile
    import concourse.mybir as mybir

    f32 = mybir.dt.float32
    bf16 = mybir.dt.bfloat16

    nc = bass.Bass("TRN2", debug=False, num_devices=8)

    xt_d = nc.dram_tensor("xt", [128, 4, 8, 512], bf16, kind="ExternalInput").ap()
    wq_d = nc.dram_tensor("wq", [128, 8, CG], bf16, kind="ExternalInput").ap()
    wk_d = nc.dram_tensor("wk", [128, 8, CG], bf16, kind="ExternalInput").ap()
    wv_d = nc.dram_tensor("wv", [128, 8, CG], bf16, kind="ExternalInput").ap()
    wp_d = nc.dram_tensor("wp", [128, 2, C], bf16, kind="ExternalInput").ap()
    bq_d = nc.dram_tensor("bq2", [128, 2], f32, kind="ExternalInput").ap()
    bk_d = nc.dram_tensor("bk2", [128, 2], f32, kind="ExternalInput").ap()
    bvb_d = nc.dram_tensor("bvb", [128, HG, D], f32, kind="ExternalInput").ap()
    cm_d = nc.dram_tensor("cmask", [128, 3, 512], bf16, kind="ExternalInput").ap()
    gs_d = nc.dram_tensor("gsel", [128, 6, 128], bf16, kind="ExternalInput").ap()
    out_d = nc.dram_tensor("out", [T, C], bf16, kind="ExternalOutput").ap()

    Exp = mybir.ActivationFunctionType.Exp
    MULT = mybir.AluOpType.mult

    with tile.TileContext(nc) as tc, \
         tc.tile_pool(name="persist", bufs=1) as persist, \
         tc.tile_pool(name="work", bufs=1) as work, \
         tc.tile_pool(name="ps_a", space="PSUM", bufs=2) as ps_a, \
         tc.tile_pool(name="ps_s", space="PSUM", bufs=2) as ps_s, \
         tc.tile_pool(name="ps_y", space="PSUM", bufs=2) as ps_y:
        # ---------- persistent SBUF ----------
        qt = [persist.tile([128, T], bf16, name=f"qt{m}", tag=f"qt{m}") for m in range(2)]
        kt = [persist.tile([128, TK], bf16, name=f"kt{m}", tag=f"kt{m}") for m in range(2)]
        vsb = persist.tile([128, NTK, HG, 2 * D], bf16, name="vsb", tag="vsb")
        yt = [persist.tile([128, T], bf16, name=f"yt{m}", tag=f"yt{m}") for m in range(2)]
        cmask = persist.tile([128, 3, 512], bf16, name="cmask", tag="cmask")
        bqs = persist.tile([128, 2], f32, name="bqs", tag="bqs")
        bks = persist.tile([128, 2], f32, name="bks", tag="bks")
        bvb = persist.tile([128, HG, D], f32, name="bvb", tag="bvb")
        bvf = bvb[:].rearrange("p h d -> p (h d)")
        wp_t = persist.tile([128, 2, C], bf16, name="wp_t", tag="wp_t")
        gsel = persist.tile([128, 6, 128], bf16, name="gsel", tag="gsel")
        wq_t = persist.tile([128, 8, CG], bf16, name="wq_t", tag="wq_t")
        wk_t = persist.tile([128, 8, CG], bf16, name="wk_t", tag="wk_t")
        wv_t = persist.tile([128, 8, CG], bf16, name="wv_t", tag="wv_t")
        xt = [persist.tile([128, 8, 512], bf16, name=f"x{j}", tag=f"xtw{j}")
              for j in range(4)]

        # ---------- DMA emission, first-use order ----------
        # ones block of vsb generated on device (no DMA)
        for i in range(NTK):
            for h in range(HG):
                nc.gpsimd.memset(vsb[:, i, h, D:2 * D], 1.0)
        nc.gpsimd.dma_start(bvb[:], bvb_d[:])
        # scalar HWDGE ring: first wq chunk, then the rest in first-use order
        nc.scalar.dma_start(wq_t[:, 0:2, :], wq_d[:, 0:2, :])
        nc.scalar.dma_start(wq_t[:, 2:8, :], wq_d[:, 2:8, :])
        # sync HWDGE ring: x window 0 k-granular so QKV j=0 starts ASAP
        for k in range(8):
            nc.sync.dma_start(xt[0][:, k, :], xt_d[:, 0, k, :])
        for h in range(2):
            nc.scalar.dma_start(wk_t[:, 4 * h:4 * h + 4, :],
                                wk_d[:, 4 * h:4 * h + 4, :])
        for h in range(2):
            nc.sync.dma_start(xt[1][:, 4 * h:4 * h + 4, :],
                              xt_d[:, 1, 4 * h:4 * h + 4, :])
        nc.scalar.dma_start(wv_t[:], wv_d[:])
        nc.scalar.dma_start(bqs[:], bq_d[:])
        nc.scalar.dma_start(bks[:], bk_d[:])
        nc.scalar.dma_start(cmask[:], cm_d[:])
        nc.scalar.dma_start(gsel[:], gs_d[:])
        nc.scalar.dma_start(wp_t[:], wp_d[:])
        # x windows 2,3 on the scalar ring (tiles are persistent: no slot waits)
        for j in range(2, 4):
            nc.scalar.dma_start(xt[j][:], xt_d[:, j, :, :])


        # ---- PE warm-up: dummy matmuls ramp HAM to full clock while the
        # first x/wq DMAs are in flight (PE would otherwise start at 1.2GHz)
        wscr = persist.tile([128, 512], bf16, name="wscr", tag="wscr")
        wsink = persist.tile([128, 4], f32, name="wsink", tag="wsink")
        nc.vector.memset(wscr[:], 0.0)
        pw = ps_y.tile([128, 512], f32, name="pwarm", tag="pyo")
        for w in range(11):
            nc.tensor.matmul(pw[:], wscr[:, 0:128], wscr[:], start=True, stop=True)
        # reader so the pool slot is released before attention needs it
        nc.vector.tensor_copy(wsink[:], pw[:, 0:4])

        for j in range(4):
            # ---- QKV for this column window ----
            # j=0 runs k-outer so matmuls start as soon as the first x k-chunk
            # and wq chunk land (DMA-paced warmup); later windows run m-outer
            # (no PSUM-slot stall between Q/K/V phases).
            if j == 0:
                pqs = [ps_a.tile([128, 512], f32, name=f"pq0_{m}", tag="acc")
                       for m in range(2)]
                for k in range(8):
                    for m in range(2):
                        nc.tensor.matmul(pqs[m][:],
                                         wq_t[:, k, 128 * m:128 * (m + 1)],
                                         xt[0][:, k, :],
                                         start=(k == 0), stop=(k == 7))
                for m in range(2):
                    nc.vector.tensor_scalar_add(qt[m][:, 0:512],
                                                pqs[m][:], bqs[:, m:m + 1])
                pks = [ps_a.tile([128, 512], f32, name=f"pk0_{m}", tag="acc")
                       for m in range(2)]
                for k in range(8):
                    for m in range(2):
                        nc.tensor.matmul(pks[m][:],
                                         wk_t[:, k, 128 * m:128 * (m + 1)],
                                         xt[0][:, k, :],
                                         start=(k == 0), stop=(k == 7))
                for m in range(2):
                    pkc = pks[m][:].rearrange("p (a b) -> p a b", b=4)[:, :, 0:3]
                    nc.vector.tensor_scalar_add(kt[m][:, 0:384],
                                                pkc, bks[:, m:m + 1])
            else:
                for m in range(2):
                    pq = ps_a.tile([128, 512], f32, tag="acc")
                    for k in range(8):
                        nc.tensor.matmul(pq[:], wq_t[:, k, 128 * m:128 * (m + 1)],
                                         xt[j][:, k, :], start=(k == 0), stop=(k == 7))
                    nc.vector.tensor_scalar_add(qt[m][:, 512 * j:512 * (j + 1)],
                                                pq[:], bqs[:, m:m + 1])
                for m in range(2):
                    pk = ps_a.tile([128, 512], f32, tag="acc")
                    for k in range(8):
                        nc.tensor.matmul(pk[:], wk_t[:, k, 128 * m:128 * (m + 1)],
                                         xt[j][:, k, :], start=(k == 0), stop=(k == 7))
                    # compact to kept key columns (drop t%4==3) during the copy
                    pkc = pk[:].rearrange("p (a b) -> p a b", b=4)[:, :, 0:3]
                    nc.vector.tensor_scalar_add(kt[m][:, 384 * j:384 * (j + 1)],
                                                pkc, bks[:, m:m + 1])
            vfull = []
            for mm in range(4):
                pv = ps_a.tile([128, 512], f32, tag="acc")
                for k in range(8):
                    nc.tensor.matmul(pv[:, 0:CG],
                                     xt[j][:, k, 128 * mm:128 * (mm + 1)],
                                     wv_t[:, k, :], start=(k == 0), stop=(k == 7))
                vf = work.tile([128, CG], bf16, name=f"vf{j}_{mm}", tag="vf",
                               bufs=6)
                nc.vector.scalar_tensor_tensor(
                    out=vf[:], in0=pv[:, 0:CG], scalar=1.0, in1=bvf[:],
                    op0=mybir.AluOpType.bypass, op1=mybir.AluOpType.add)
                vfull.append(vf)
            # gather kept V rows (partition gather via 0/1 selection matmuls)
            for s in range(3):
                i = 3 * j + s
                pvk = ps_a.tile([128, 512], f32, tag="acc")
                nc.tensor.matmul(pvk[:, 0:CG], gsel[:, 2 * s, :], vfull[s][:],
                                 start=True, stop=False)
                nc.tensor.matmul(pvk[:, 0:CG], gsel[:, 2 * s + 1, :], vfull[s + 1][:],
                                 start=False, stop=True)
                nc.vector.tensor_copy(
                    vsb[:, i, :, 0:D],
                    pvk[:, 0:CG].rearrange("p (h d) -> p h d", d=D))

            # ---- attention: head pairs share the PE via row groups 0/64 ----
            # Score pairs are emitted adjacently; AV of tile i-1 comes after
            # the score pair of tile i so the pairs fuse (concurrent row
            # groups) in the static schedule.
            jwin = slice(512 * j, 512 * (j + 1))
            ntile = 3 * (j + 1)
            nb0 = ntile - 3  # first boundary tile index
            pycps = []  # SBUF copies of the AV accumulators, per (hp, q)
            for hp in range(2):
                pys = [ps_y.tile([128, 512], f32, name=f"py{hp}_{q}", tag="pyo")
                       for q in range(2)]
                pend = None  # (i, pt2) awaiting AV emission
                for i in range(ntile):
                    ps2 = ps_s.tile([128, 2, 512], f32, tag="ps2")
                    pt2 = work.tile([128, 2, 512], bf16, tag="pt2", bufs=4)
                    u = i - nb0
                    # boundary tiles u=1,2: cols [0:off) are fully masked —
                    # skip computing them (the mask-mult zeroes the stale data)
                    off = (0, 128, 320)[u] if u >= 1 else 0
                    with tc.high_priority():
                        for q in range(2):  # q: row group (head 2*hp + q)
                            nc.tensor.matmul(
                                ps2[:, q, off:512],
                                kt[hp][64 * q:64 * q + 64, 128 * i:128 * (i + 1)],
                                qt[hp][64 * q:64 * q + 64,
                                       512 * j + off:512 * (j + 1)],
                                start=True, stop=True)
                    nc.scalar.activation(pt2[:, :, off:512], ps2[:, :, off:512],
                                         Exp, bias=0.0, scale=SCALE)
                    if off:  # zero the skipped (fully masked) columns
                        nc.gpsimd.memset(pt2[:, :, 0:off], 0.0)
                    if u >= 0:  # boundary tile: causal mask (both heads)
                        w = (192, 384, 512)[u]
                        for q in range(2):
                            nc.vector.tensor_tensor(
                                pt2[:, q, off:w], pt2[:, q, off:w],
                                cmask[:, u, off:w], op=MULT)
                    if pend is not None:
                        pi, ppt = pend
                        for q in range(2):
                            nc.tensor.matmul(
                                pys[q][:], vsb[:, pi, 2 * hp + q, :],
                                ppt[:, q, :],
                                start=(pi == 0), stop=(pi == ntile - 1))
                    pend = (i, pt2)
                pi, ppt = pend
                for q in range(2):
                    nc.tensor.matmul(
                        pys[q][:], vsb[:, pi, 2 * hp + q, :], ppt[:, q, :],
                        start=(pi == 0), stop=(pi == ntile - 1))
                if j < 3 or hp == 0:
                    # free the PSUM slots fast (AV of the next hp needs them):
                    # copy the accumulators to SBUF; normalize later from there
                    for q in range(2):
                        pycp = work.tile([128, 512], f32, name=f"pycp{hp}_{q}",
                                         tag="pycp", bufs=4)
                        nc.vector.tensor_copy(pycp[:], pys[q][:])
                        pycps.append((hp, q, pycp))
                else:
                    pys_last = pys
            # ---- softmax normalization, off the AV critical path ----
            for hp, q, pycp in pycps:
                rec = work.tile([64, 512], f32, tag="rec", bufs=2)
                lns = work.tile([64, 512], f32, tag="lns", bufs=2)
                nc.scalar.activation(lns[:], pycp[64:128, :],
                                     mybir.ActivationFunctionType.Ln)
                nc.scalar.activation(rec[:], lns[:], Exp, bias=0.0,
                                     scale=-1.0)
                nc.vector.tensor_tensor(
                    yt[hp][64 * q:64 * q + 64, jwin],
                    pycp[0:64, :], rec[:], op=MULT)

            # ---- output projection for the finished query window ----
            if j < 3:
                for m in range(4 * j, 4 * j + 4):
                    stage = work.tile([128, C], bf16, tag="stage", bufs=2)
                    for n in range(2):
                        po = ps_y.tile([128, 512], f32, tag="pyo")
                        for k2 in range(2):
                            nc.tensor.matmul(
                                po[:], yt[k2][:, 128 * m:128 * (m + 1)],
                                wp_t[:, k2, 512 * n:512 * (n + 1)],
                                start=(k2 == 0), stop=(k2 == 1))
                        nc.vector.tensor_copy(stage[:, 512 * n:512 * (n + 1)], po[:])
                    nc.sync.dma_start(out_d[128 * m:128 * (m + 1), :], stage[:])
            else:
                # ---- j=3 endgame: minimize the tail chain ----
                # hp1 normalizes straight from PSUM; yt chunks written per
                # 128-query m-chunk so each proj tile starts immediately.
                # Proj PSUM comes from ps_a (idle after the last QKV) so it
                # does not wait on the pys slots still being read.
                recs = []
                for q in range(2):
                    rec = work.tile([64, 512], f32, name=f"recL{q}",
                                    tag="rec", bufs=2)
                    lns = work.tile([64, 512], f32, name=f"lnsL{q}",
                                    tag="lns", bufs=2)
                    nc.scalar.activation(lns[:], pys_last[q][64:128, :],
                                         mybir.ActivationFunctionType.Ln)
                    nc.scalar.activation(rec[:], lns[:], Exp, bias=0.0,
                                         scale=-1.0)
                    recs.append(rec)
                for m in range(12, 16):
                    mc = 128 * (m - 12)
                    for q in range(2):
                        nc.vector.tensor_tensor(
                            yt[1][64 * q:64 * q + 64, 128 * m:128 * (m + 1)],
                            pys_last[q][0:64, mc:mc + 128],
                            recs[q][:, mc:mc + 128], op=MULT)
                    stage = work.tile([128, C], bf16, tag="stage", bufs=2)
                    for n in range(2):
                        po = ps_a.tile([128, 512], f32, name=f"poL{m}_{n}",
                                       tag="acc")
                        for k2 in range(2):
                            nc.tensor.matmul(
                                po[:], yt[k2][:, 128 * m:128 * (m + 1)],
                                wp_t[:, k2, 512 * n:512 * (n + 1)],
                                start=(k2 == 0), stop=(k2 == 1))
                        nc.vector.tensor_copy(stage[:, 512 * n:512 * (n + 1)],
                                              po[:])
                        nc.sync.dma_start(
                            out_d[128 * m:128 * (m + 1), 512 * n:512 * (n + 1)],
                            stage[:, 512 * n:512 * (n + 1)])

    if split:
        _split_multi_waits(nc, mybir)
    return nc


def _get_nc():
    if "nc" not in _CACHE:
        _CACHE["nc"] = _build_bass()
    return _CACHE["nc"]


def _host_maps(inputs):
    x = np.asarray(inputs["x"], np.float32)
    Wq = np.asarray(inputs["Wq"], np.float32)
    Wk = np.asarray(inputs["Wk"], np.float32)
    Wv = np.asarray(inputs["Wv"], np.float32)
    Wp = np.asarray(inputs["Wp"], np.float32)
    bq = np.asarray(inputs["bq"], np.float32)
    bk = np.asarray(inputs["bk"], np.float32)
    bv = np.asarray(inputs["bv"], np.float32)

    # causal masks in compacted key coordinates: 3 boundary chunks
    p = np.arange(128)
    f = np.arange(512)
    cm = np.zeros((128, 3, 512), np.float32)
    for u in range(3):
        q = 128 * u + p
        g = (q // 3) * 4 + (q % 3)
        cm[:, u, :] = (f[None, :] >= g[:, None]).astype(np.float32)

    # V row-gather selection matrices: kept chunk i = 3k+s draws rows from
    # original chunks 4k+s and 4k+s+1; G[s][side][p, m] = 1 iff kept row m
    # maps to row p of that original chunk.
    gs = np.zeros((128, 6, 128), np.float32)
    for s in range(3):
        for m in range(128):
            orr = ((128 * s + m) // 3) * 4 + (128 * s + m) % 3
            side = 0 if orr < 128 * (s + 1) else 1
            gs[orr - 128 * (s + side), 2 * s + side, m] = 1.0

    def packw(w):  # [1024, n] -> [128, 8, n] partition-major
        return np.ascontiguousarray(
            w.reshape(8, 128, -1).transpose(1, 0, 2)).astype(ml_dtypes.bfloat16)

    # x[b].T packed as [128, j, k, 512]: xh[p, j, k, t] = x[b][j*512+t, k*128+p]
    xhs = []
    for b in range(B):
        xt = x[b].T  # [C, T]
        xh = xt.reshape(8, 128, 4, 512).transpose(1, 2, 0, 3)
        xhs.append(np.ascontiguousarray(xh).astype(ml_dtypes.bfloat16))

    maps = []
    for c in range(8):
        b, g = c // 4, c % 4
        sl = slice(CG * g, CG * (g + 1))
        maps.append({
            "xt": xhs[b],
            "wq": packw(Wq[:, sl]),
            "wk": packw(Wk[:, sl]),
            "wv": packw(Wv[:, sl]),
            "wp": np.ascontiguousarray(
                Wp[sl, :].reshape(2, 128, C).transpose(1, 0, 2)
            ).astype(ml_dtypes.bfloat16),
            "bq2": np.ascontiguousarray(bq[sl].reshape(2, 128).T),
            "bk2": np.ascontiguousarray(bk[sl].reshape(2, 128).T),
            "bvb": np.ascontiguousarray(
                np.broadcast_to(bv[sl].reshape(HG, D), (128, HG, D))),
            "cmask": cm.astype(ml_dtypes.bfloat16),
            "gsel": gs.astype(ml_dtypes.bfloat16),
        })
    return maps


def _combine(results, inputs):
    bp = np.asarray(inputs["bp"], np.float32)
    out = np.zeros((B, T, C), np.float32)
    for c in range(8):
        out[c // 4] += np.asarray(results[c]["out"], np.float32)
    out += bp[None, None, :]
    return out


def _run(inputs, profile_dir=None, trace_cores=None):
    nc = _get_nc()
    maps = _host_maps(inputs)
    from concourse.bass_utils import run_bass_kernel_spmd
    if profile_dir is not None:
        import types, sys
        from trn_agent_boot.trn_boot import _ntff_profile_via_ctypes
        hook = _ntff_profile_via_ctypes("/opt/axon/libaxon_pjrt.so")
        with hook(profile_dir, trace_cores or [0]):
            res = run_bass_kernel_spmd(nc, maps, core_ids=list(range(8)))
    else:
        res = run_bass_kernel_spmd(nc, maps, core_ids=list(range(8)))
    return _combine(res.results, inputs)


def kernel(**inputs):
    return _run(inputs)


# revision 19
# speedup vs baseline: 1.0085x; 1.0085x over previous
# Trainium2 Bass kernel for CausalSelfAttention (B=2, T=2048, C=1024, H=16, D=64)
# with periodic mask: causal AND (key_col % 4 != 3).
#
# Sharding (8 NeuronCores): core c = (b, g) with b = c//4 (batch), g = c%4
# (head group of 4 heads). Each core computes QKV for its 4 heads, attention,
# and a partial output projection y_heads @ Wp[rows]. Host sums the 4 partials
# per batch and adds bp (tensor-parallel reduce).
#
# Key device-side choices:
#  - All DRAM inputs are host-prepacked to [128, ...] partition-major layouts
#    so every DMA descriptor moves 4-8KB contiguous runs (fewer descriptors,
#    less issuing-engine busy time, faster rings).
#  - DMAs are emitted in first-use order, k-granular for the j=0 window, so
#    the first QKV matmul starts as early as possible and the PE never goes
#    HAM-cold once started.
#  - The periodic mask is exploited as compaction: keys at t%4==3 are never
#    attended. K^T is computed directly at kept columns via a 3D moving AP
#    (384-wide matmuls); V is computed full then row-gathered via 0/1
#    selection matmuls.
#  - Scores are produced transposed (S^T[tk_kept, tq]); the two heads of a
#    row-group pair are issued back-to-back so their K=64 matmuls execute
#    CONCURRENTLY on disjoint PE row groups (h0/h64). AV of tile i-1 is
#    emitted after the score pair of tile i to keep pairs adjacent in the
#    static schedule.
#  - Softmax row sums: V tiles carry a 64-wide all-ones block (memset on
#    device), so each AV accumulation produces [64 x y ; 64 x row-sums] in
#    PSUM; a DVE fast-reciprocal + multiply normalizes during PSUM->SBUF.
#  - The projection path (yt, Wp, stage, out) runs in bf16: halves the
#    output DMA and copy costs at ~1e-3 extra relative error.

import ml_dtypes
import numpy as np

B, T, C, H, D = 2, 2048, 1024, 16, 64
HG = 4          # heads per core
CG = HG * D     # = 256 columns of C per core
TK = (T // 4) * 3   # 1536 kept key positions
NTK = TK // 128     # 12 kept-key chunks of 128
SCALE = 1.0 / 8.0   # 1/sqrt(D)

_CACHE = {}


def _split_multi_waits(nc, mybir):
    # The pinned walrus here encodes at most 1 sync-wait per instruction
    # (2 for EventSemaphore). Hoist excess waits onto standalone NoOps that
    # precede the instruction on the same engine.
    f = nc.m.functions[0]
    n = 0
    for b in f.blocks:
        insts = list(b.instructions)
        out = []
        changed = False
        for inst in insts:
            si = inst.sync_info
            if si is not None:
                waits = list(si.on_wait)
                cap = 2 if isinstance(inst, mybir.InstEventSemaphore) else 1
                if len(waits) > cap:
                    for w in waits[cap:]:
                        out.append(mybir.InstNoOp(
                            name=f"{inst.name}-ws{n}", engine=inst.engine,
                            ins=[], outs=[],
                            sync_info=mybir.SyncInfo(on_wait=[w], on_update=[])))
                        n += 1
                    inst.sync_info = mybir.SyncInfo(
                        on_wait=waits[:cap], on_update=list(si.on_update))
                    changed = True
            out.append(inst)
        if changed:
            b.instructions = out
    return n


def _build_bass(split=True):
    import concourse.bass as bass
    import concourse.tile as tile
    import concourse.mybir as mybir

    f32 = mybir.dt.float32
    bf16 = mybir.dt.bfloat16

    nc = bass.Bass("TRN2", debug=False, num_devices=8)

    xt_d = nc.dram_tensor("xt", [128, 4, 8, 512], bf16, kind="ExternalInput").ap()
    wq_d = nc.dram_tensor("wq", [128, 8, CG], bf16, kind="ExternalInput").ap()
    wk_d = nc.dram_tensor("wk", [128, 8, CG], bf16, kind="ExternalInput").ap()
    wv_d = nc.dram_tensor("wv", [128, 8, CG], bf16, kind="ExternalInput").ap()
    wp_d = nc.dram_tensor("wp", [128, 2, C], bf16, kind="ExternalInput").ap()
    bq_d = nc.dram_tensor("bq2", [128, 2], f32, kind="ExternalInput").ap()
    bk_d = nc.dram_tensor("bk2", [128, 2], f32, kind="ExternalInput").ap()
    bvb_d = nc.dram_tensor("bvb", [128, HG, D], f32, kind="ExternalInput").ap()
    cm_d = nc.dram_tensor("cmask", [128, 3, 512], bf16, kind="ExternalInput").ap()
    gs_d = nc.dram_tensor("gsel", [128, 6, 128], bf16, kind="ExternalInput").ap()
    out_d = nc.dram_tensor("out", [T, C], bf16, kind="ExternalOutput").ap()

    Exp = mybir.ActivationFunctionType.Exp
    MULT = mybir.AluOpType.mult

    with tile.TileContext(nc) as tc, \
         tc.tile_pool(name="persist", bufs=1) as persist, \
         tc.tile_pool(name="work", bufs=1) as work, \
         tc.tile_pool(name="ps_a", space="PSUM", bufs=2) as ps_a, \
         tc.tile_pool(name="ps_s", space="PSUM", bufs=2) as ps_s, \
         tc.tile_pool(name="ps_y", space="PSUM", bufs=2) as ps_y:
        # ---------- persistent SBUF ----------
        qt = [persist.tile([128, T], bf16, name=f"qt{m}", tag=f"qt{m}") for m in range(2)]
        kt = [persist.tile([128, TK], bf16, name=f"kt{m}", tag=f"kt{m}") for m in range(2)]
        vsb = persist.tile([128, NTK, HG, 2 * D], bf16, name="vsb", tag="vsb")
        yt = [persist.tile([128, T], bf16, name=f"yt{m}", tag=f"yt{m}") for m in range(2)]
        cmask = persist.tile([128, 3, 512], bf16, name="cmask", tag="cmask")
        bqs = persist.tile([128, 2], f32, name="bqs", tag="bqs")
        bks = persist.tile([128, 2], f32, name="bks", tag="bks")
        bvb = persist.tile([128, HG, D], f32, name="bvb", tag="bvb")
        bvf = bvb[:].rearrange("p h d -> p (h d)")
        wp_t = persist.tile([128, 2, C], bf16, name="wp_t", tag="wp_t")
        gsel = persist.tile([128, 6, 128], bf16, name="gsel", tag="gsel")
        wq_t = persist.tile([128, 8, CG], bf16, name="wq_t", tag="wq_t")
        wk_t = persist.tile([128, 8, CG], bf16, name="wk_t", tag="wk_t")
        wv_t = persist.tile([128, 8, CG], bf16, name="wv_t", tag="wv_t")
        xt = [persist.tile([128, 8, 512], bf16, name=f"x{j}", tag=f"xtw{j}")
              for j in range(4)]

        # ---------- DMA emission, first-use order ----------
        # ones block of vsb generated on device (no DMA)
        for i in range(NTK):
            for h in range(HG):
                nc.gpsimd.memset(vsb[:, i, h, D:2 * D], 1.0)
        nc.gpsimd.dma_start(bvb[:], bvb_d[:])
        # scalar HWDGE ring: first wq chunk, then the rest in first-use order
        nc.scalar.dma_start(wq_t[:, 0:2, :], wq_d[:, 0:2, :])
        nc.scalar.dma_start(wq_t[:, 2:8, :], wq_d[:, 2:8, :])
        # sync HWDGE ring: x window 0 k-granular so QKV j=0 starts ASAP
        for k in range(8):
            nc.sync.dma_start(xt[0][:, k, :], xt_d[:, 0, k, :])
        for h in range(2):
            nc.scalar.dma_start(wk_t[:, 4 * h:4 * h + 4, :],
                                wk_d[:, 4 * h:4 * h + 4, :])
        for h in range(2):
            nc.sync.dma_start(xt[1][:, 4 * h:4 * h + 4, :],
                              xt_d[:, 1, 4 * h:4 * h + 4, :])
        nc.scalar.dma_start(wv_t[:], wv_d[:])
        nc.scalar.dma_start(bqs[:], bq_d[:])
        nc.scalar.dma_start(bks[:], bk_d[:])
        nc.scalar.dma_start(cmask[:], cm_d[:])
        nc.scalar.dma_start(gsel[:], gs_d[:])
        nc.scalar.dma_start(wp_t[:], wp_d[:])
        # x windows 2,3 on the scalar ring (tiles are persistent: no slot waits)
        for j in range(2, 4):
            nc.scalar.dma_start(xt[j][:], xt_d[:, j, :, :])


        # ---- PE warm-up: dummy matmuls ramp HAM to full clock while the
        # first x/wq DMAs are in flight (PE would otherwise start at 1.2GHz)
        wscr = persist.tile([128, 512], bf16, name="wscr", tag="wscr")
        wsink = persist.tile([128, 4], f32, name="wsink", tag="wsink")
        nc.vector.memset(wscr[:], 0.0)
        pw = ps_y.tile([128, 512], f32, name="pwarm", tag="pyo")
        for w in range(5):
            nc.tensor.matmul(pw[:], wscr[:, 0:128], wscr[:], start=True, stop=True)
        # reader so the pool slot is released before attention needs it
        nc.vector.tensor_copy(wsink[:], pw[:, 0:4])

        for j in range(4):
            # ---- QKV for this column window ----
            # j=0 runs k-outer so matmuls start as soon as the first x k-chunk
            # and wq chunk land (DMA-paced warmup); later windows run m-outer
            # (no PSUM-slot stall between Q/K/V phases).
            if j == 0:
                pqs = [ps_a.tile([128, 512], f32, name=f"pq0_{m}", tag="acc")
                       for m in range(2)]
                for k in range(8):
                    for m in range(2):
                        nc.tensor.matmul(pqs[m][:],
                                         wq_t[:, k, 128 * m:128 * (m + 1)],
                                         xt[0][:, k, :],
                                         start=(k == 0), stop=(k == 7))
                for m in range(2):
                    nc.vector.tensor_scalar_add(qt[m][:, 0:512],
                                                pqs[m][:], bqs[:, m:m + 1])
                pks = [ps_a.tile([128, 512], f32, name=f"pk0_{m}", tag="acc")
                       for m in range(2)]
                for k in range(8):
                    for m in range(2):
                        nc.tensor.matmul(pks[m][:],
                                         wk_t[:, k, 128 * m:128 * (m + 1)],
                                         xt[0][:, k, :],
                                         start=(k == 0), stop=(k == 7))
                for m in range(2):
                    pkc = pks[m][:].rearrange("p (a b) -> p a b", b=4)[:, :, 0:3]
                    nc.vector.tensor_scalar_add(kt[m][:, 0:384],
                                                pkc, bks[:, m:m + 1])
            else:
                for m in range(2):
                    pq = ps_a.tile([128, 512], f32, tag="acc")
                    for k in range(8):
                        nc.tensor.matmul(pq[:], wq_t[:, k, 128 * m:128 * (m + 1)],
                                         xt[j][:, k, :], start=(k == 0), stop=(k == 7))
                    nc.vector.tensor_scalar_add(qt[m][:, 512 * j:512 * (j + 1)],
                                                pq[:], bqs[:, m:m + 1])
                for m in range(2):
                    pk = ps_a.tile([128, 512], f32, tag="acc")
                    for k in range(8):
                        nc.tensor.matmul(pk[:], wk_t[:, k, 128 * m:128 * (m + 1)],
                                         xt[j][:, k, :], start=(k == 0), stop=(k == 7))
                    # compact to kept key columns (drop t%4==3) during the copy
                    pkc = pk[:].rearrange("p (a b) -> p a b", b=4)[:, :, 0:3]
                    nc.vector.tensor_scalar_add(kt[m][:, 384 * j:384 * (j + 1)],
                                                pkc, bks[:, m:m + 1])
            vfull = []
            for mm in range(4):
                pv = ps_a.tile([128, 512], f32, tag="acc")
                for k in range(8):
                    nc.tensor.matmul(pv[:, 0:CG],
                                     xt[j][:, k, 128 * mm:128 * (mm + 1)],
                                     wv_t[:, k, :], start=(k == 0), stop=(k == 7))
                vf = work.tile([128, CG], bf16, name=f"vf{j}_{mm}", tag="vf",
                               bufs=6)
                nc.vector.scalar_tensor_tensor(
                    out=vf[:], in0=pv[:, 0:CG], scalar=1.0, in1=bvf[:],
                    op0=mybir.AluOpType.bypass, op1=mybir.AluOpType.add)
                vfull.append(vf)
            # gather kept V rows (partition gather via 0/1 selection matmuls)
            for s in range(3):
                i = 3 * j + s
                pvk = ps_a.tile([128, 512], f32, tag="acc")
                nc.tensor.matmul(pvk[:, 0:CG], gsel[:, 2 * s, :], vfull[s][:],
                                 start=True, stop=False)
                nc.tensor.matmul(pvk[:, 0:CG], gsel[:, 2 * s + 1, :], vfull[s + 1][:],
                                 start=False, stop=True)
                nc.vector.tensor_copy(
                    vsb[:, i, :, 0:D],
                    pvk[:, 0:CG].rearrange("p (h d) -> p h d", d=D))

            # ---- attention: head pairs share the PE via row groups 0/64 ----
            # Score pairs are emitted adjacently; AV of tile i-1 comes after
            # the score pair of tile i so the pairs fuse (concurrent row
            # groups) in the static schedule.
            jwin = slice(512 * j, 512 * (j + 1))
            ntile = 3 * (j + 1)
            nb0 = ntile - 3  # first boundary tile index
            pycps = []  # SBUF copies of the AV accumulators, per (hp, q)
            for hp in range(2):
                pys = [ps_y.tile([128, 512], f32, name=f"py{hp}_{q}", tag="pyo")
                       for q in range(2)]
                pend = None  # (i, pt2) awaiting AV emission
                for i in range(ntile):
                    ps2 = ps_s.tile([128, 2, 512], f32, tag="ps2")
                    pt2 = work.tile([128, 2, 512], bf16, tag="pt2", bufs=4)
                    u = i - nb0
                    # boundary tiles u=1,2: cols [0:off) are fully masked —
                    # skip computing them (the mask-mult zeroes the stale data)
                    off = (0, 128, 320)[u] if u >= 1 else 0
                    with tc.high_priority():
                        for q in range(2):  # q: row group (head 2*hp + q)
                            nc.tensor.matmul(
                                ps2[:, q, off:512],
                                kt[hp][64 * q:64 * q + 64, 128 * i:128 * (i + 1)],
                                qt[hp][64 * q:64 * q + 64,
                                       512 * j + off:512 * (j + 1)],
                                start=True, stop=True)
                    nc.scalar.activation(pt2[:, :, off:512], ps2[:, :, off:512],
                                         Exp, bias=0.0, scale=SCALE)
                    if off:  # zero the skipped (fully masked) columns
                        nc.gpsimd.memset(pt2[:, :, 0:off], 0.0)
                    if u >= 0:  # boundary tile: causal mask (both heads)
                        w = (192, 384, 512)[u]
                        for q in range(2):
                            nc.vector.tensor_tensor(
                                pt2[:, q, off:w], pt2[:, q, off:w],
                                cmask[:, u, off:w], op=MULT)
                    if pend is not None:
                        pi, ppt = pend
                        for q in range(2):
                            nc.tensor.matmul(
                                pys[q][:], vsb[:, pi, 2 * hp + q, :],
                                ppt[:, q, :],
                                start=(pi == 0), stop=(pi == ntile - 1))
                    pend = (i, pt2)
                pi, ppt = pend
                for q in range(2):
                    nc.tensor.matmul(
                        pys[q][:], vsb[:, pi, 2 * hp + q, :], ppt[:, q, :],
                        start=(pi == 0), stop=(pi == ntile - 1))
                if j < 3 or hp == 0:
                    # free the PSUM slots fast (AV of the next hp needs them):
                    # copy the accumulators to SBUF; normalize later from there
                    pycp = work.tile([128, 2, 512], f32, name=f"pycp{hp}",
                                     tag="pycp", bufs=2)
                    for q in range(2):
                        nc.vector.tensor_copy(pycp[:, q, :], pys[q][:])
                    pycps.append((hp, pycp))
                else:
                    pys_last = pys
            # ---- softmax normalization, off the AV critical path ----
            # q0/q1 merged: one Ln + one Exp over [64, 1024]
            for hp, pycp in pycps:
                rec = work.tile([64, 2, 512], f32, tag="rec", bufs=2)
                lns = work.tile([64, 2, 512], f32, tag="lns", bufs=2)
                nc.scalar.activation(lns[:], pycp[64:128, :, :],
                                     mybir.ActivationFunctionType.Ln)
                nc.scalar.activation(rec[:], lns[:], Exp, bias=0.0,
                                     scale=-1.0)
                for q in range(2):
                    nc.vector.tensor_tensor(
                        yt[hp][64 * q:64 * q + 64, jwin],
                        pycp[0:64, q, :], rec[:, q, :], op=MULT)

            # ---- output projection for the finished query window ----
            if j < 3:
                for m in range(4 * j, 4 * j + 4):
                    stage = work.tile([128, C], bf16, tag="stage", bufs=2)
                    for n in range(2):
                        po = ps_y.tile([128, 512], f32, tag="pyo")
                        for k2 in range(2):
                            nc.tensor.matmul(
                                po[:], yt[k2][:, 128 * m:128 * (m + 1)],
                                wp_t[:, k2, 512 * n:512 * (n + 1)],
                                start=(k2 == 0), stop=(k2 == 1))
                        nc.vector.tensor_copy(stage[:, 512 * n:512 * (n + 1)], po[:])
                    nc.sync.dma_start(out_d[128 * m:128 * (m + 1), :], stage[:])
            else:
                # ---- j=3 endgame: minimize the tail chain ----
                # hp1 normalizes straight from PSUM; yt chunks written per
                # 128-query m-chunk so each proj tile starts immediately.
                # Proj PSUM comes from ps_a (idle after the last QKV) so it
                # does not wait on the pys slots still being read.
                recs = []
                for q in range(2):
                    rec = work.tile([64, 512], f32, name=f"recL{q}",
                                    tag="rec", bufs=2)
                    lns = work.tile([64, 512], f32, name=f"lnsL{q}",
                                    tag="lns", bufs=2)
                    nc.scalar.activation(lns[:], pys_last[q][64:128, :],
                                         mybir.ActivationFunctionType.Ln)
                    nc.scalar.activation(rec[:], lns[:], Exp, bias=0.0,
                                         scale=-1.0)
                    recs.append(rec)
                for m in range(12, 16):
                    mc = 128 * (m - 12)
                    for q in range(2):
                        nc.vector.tensor_tensor(
                            yt[1][64 * q:64 * q + 64, 128 * m:128 * (m + 1)],
                            pys_last[q][0:64, mc:mc + 128],
                            recs[q][:, mc:mc + 128], op=MULT)
                    stage = work.tile([128, C], bf16, tag="stage", bufs=2)
                    for n in range(2):
                        po = ps_a.tile([128, 512], f32, name=f"poL{m}_{n}",
                                       tag="acc")
                        for k2 in range(2):
                            nc.tensor.matmul(
                                po[:], yt[k2][:, 128 * m:128 * (m + 1)],
                                wp_t[:, k2, 512 * n:512 * (n + 1)],
                                start=(k2 == 0), stop=(k2 == 1))
                        nc.vector.tensor_copy(stage[:, 512 * n:512 * (n + 1)],
                                              po[:])
                        nc.sync.dma_start(
                            out_d[128 * m:128 * (m + 1), 512 * n:512 * (n + 1)],
                            stage[:, 512 * n:512 * (n + 1)])

    if split:
        _split_multi_waits(nc, mybir)
    return nc


def _get_nc():
    if "nc" not in _CACHE:
        _CACHE["nc"] = _build_bass()
    return _CACHE["nc"]


def _host_maps(inputs):
    x = np.asarray(inputs["x"], np.float32)
    Wq = np.asarray(inputs["Wq"], np.float32)
    Wk = np.asarray(inputs["Wk"], np.float32)
    Wv = np.asarray(inputs["Wv"], np.float32)
    Wp = np.asarray(inputs["Wp"], np.float32)
    bq = np.asarray(inputs["bq"], np.float32)
    bk = np.asarray(inputs["bk"], np.float32)
    bv = np.asarray(inputs["bv"], np.float32)

    # causal masks in compacted key coordinates: 3 boundary chunks
    p = np.arange(128)
    f = np.arange(512)
    cm = np.zeros((128, 3, 512), np.float32)
    for u in range(3):
        q = 128 * u + p
        g = (q // 3) * 4 + (q % 3)
        cm[:, u, :] = (f[None, :] >= g[:, None]).astype(np.float32)

    # V row-gather selection matrices: kept chunk i = 3k+s draws rows from
    # original chunks 4k+s and 4k+s+1; G[s][side][p, m] = 1 iff kept row m
    # maps to row p of that original chunk.
    gs = np.zeros((128, 6, 128), np.float32)
    for s in range(3):
        for m in range(128):
            orr = ((128 * s + m) // 3) * 4 + (128 * s + m) % 3
            side = 0 if orr < 128 * (s + 1) else 1
            gs[orr - 128 * (s + side), 2 * s + side, m] = 1.0

    def packw(w):  # [1024, n] -> [128, 8, n] partition-major
        return np.ascontiguousarray(
            w.reshape(8, 128, -1).transpose(1, 0, 2)).astype(ml_dtypes.bfloat16)

    # x[b].T packed as [128, j, k, 512]: xh[p, j, k, t] = x[b][j*512+t, k*128+p]
    xhs = []
    for b in range(B):
        xt = x[b].T  # [C, T]
        xh = xt.reshape(8, 128, 4, 512).transpose(1, 2, 0, 3)
        xhs.append(np.ascontiguousarray(xh).astype(ml_dtypes.bfloat16))

    maps = []
    for c in range(8):
        b, g = c // 4, c % 4
        sl = slice(CG * g, CG * (g + 1))
        maps.append({
            "xt": xhs[b],
            "wq": packw(Wq[:, sl]),
            "wk": packw(Wk[:, sl]),
            "wv": packw(Wv[:, sl]),
            "wp": np.ascontiguousarray(
                Wp[sl, :].reshape(2, 128, C).transpose(1, 0, 2)
            ).astype(ml_dtypes.bfloat16),
            "bq2": np.ascontiguousarray(bq[sl].reshape(2, 128).T),
            "bk2": np.ascontiguousarray(bk[sl].reshape(2, 128).T),
            "bvb": np.ascontiguousarray(
                np.broadcast_to(bv[sl].reshape(HG, D), (128, HG, D))),
            "cmask": cm.astype(ml_dtypes.bfloat16),
            "gsel": gs.astype(ml_dtypes.bfloat16),
        })
    return maps


def _combine(results, inputs):
    bp = np.asarray(inputs["bp"], np.float32)
    out = np.zeros((B, T, C), np.float32)
    for c in range(8):
        out[c // 4] += np.asarray(results[c]["out"], np.float32)
    out += bp[None, None, :]
    return out


def _run(inputs, profile_dir=None, trace_cores=None):
    nc = _get_nc()
    maps = _host_maps(inputs)
    from concourse.bass_utils import run_bass_kernel_spmd
    if profile_dir is not None:
        import types, sys
        from trn_agent_boot.trn_boot import _ntff_profile_via_ctypes
        hook = _ntff_profile_via_ctypes("/opt/axon/libaxon_pjrt.so")
        with hook(profile_dir, trace_cores or [0]):
            res = run_bass_kernel_spmd(nc, maps, core_ids=list(range(8)))
    else:
        res = run_bass_kernel_spmd(nc, maps, core_ids=list(range(8)))
    return _combine(res.results, inputs)


def kernel(**inputs):
    return _run(inputs)
